# revision 1
# baseline (speedup 1.0000x reference)
"""ContraAttention TRN2 kernel builder (v2: fp16 matmuls + DMA transposes).

Per-core program (core i owns query batches [16i, 16i+16)):
  Qp = Xq @ Wq^T + bq ; G = Qp @ Wk ; h = Qp @ bk
  S = G @ Xk^T + h*1^T   (exact: == Qp @ (Xk Wk^T + bk)^T)
  per (a,b) 64x64 block: t2v_sum = sum_l max_m S, v2t_sum = sum_m max_l S
  r[a,b] = exp(ls) * (t2v_sum/cms[a] + v2t_sum/64) / 2

Outputs per core:
  out_t2v [16, 128]  : exp(ls)/2/cms[a] * t2v_sum  at [a_local, b]
  out_v2t [2, 1024]  : exp(ls)/128 * v2t_sum at [half, ((lc*16+mt)*4+q)*2+g]
                       contributing to a_local=2*lc+g, b=mt*8+q*2+half
"""

import sys

sys.path.insert(0, "/opt/trn_rl_repo")

import concourse.bass as bass  # noqa: F401
import concourse.mybir as mybir
import concourse.tile as tile
from concourse import bacc

F32 = mybir.dt.float32
F16 = mybir.dt.float16
AF = mybir.ActivationFunctionType
AX = mybir.AxisListType
ALU = mybir.AluOpType

N_CORES = 8
NB = 128            # global batches
AB = NB // N_CORES  # 16 batches per core
L = 64              # Lq = Lk
D = 512
LQ = AB * L         # 1024 q rows per core
MK = NB * L         # 8192 k rows
NLC = LQ // 128     # 8 l-chunks
NMT = MK // 512     # 16 m-tiles
NCC = D // 128      # 4 contraction chunks
NLT = LQ // 512     # 2 l-tiles


def build_kernel(repeat_main=1, ablate=(), transpose_mode="pe"):
    MMDT = F16

    nc = bacc.Bacc("TRN2", target_bir_lowering=False, debug=False,
                   num_devices=N_CORES)

    xq = nc.dram_tensor("xq", [LQ, D], F32, kind="ExternalInput")
    xk = nc.dram_tensor("xk", [MK, D], F32, kind="ExternalInput")
    wq = nc.dram_tensor("wq", [D, D], F32, kind="ExternalInput")
    wk = nc.dram_tensor("wk", [D, D], F32, kind="ExternalInput")
    bq4 = nc.dram_tensor("bq4", [128, NCC], F32, kind="ExternalInput")
    bk4 = nc.dram_tensor("bk4", [128, NCC], F32, kind="ExternalInput")
    mask16 = nc.dram_tensor("mask16", [AB, L], F32, kind="ExternalInput")
    ls128 = nc.dram_tensor("ls128", [128, 1], F32, kind="ExternalInput")
    ident_in = nc.dram_tensor("ident", [128, 128], F32, kind="ExternalInput")
    sel_in = nc.dram_tensor("sel", [128, 2], F32, kind="ExternalInput")
    selb_in = nc.dram_tensor("selb", [AB, NLC * 128], F32,
                             kind="ExternalInput")

    out_t2v = nc.dram_tensor("out_t2v", [AB, 128], F32, kind="ExternalOutput")
    out_v2t = nc.dram_tensor("out_v2t", [2, NLC * NMT * 8], F32,
                             kind="ExternalOutput")

    with tile.TileContext(nc) as tc:
        with (
            tc.tile_pool(name="persist", bufs=1) as pp,
            tc.tile_pool(name="stg", bufs=4) as stg,      # fp32 2KB stages
            tc.tile_pool(name="qpool", bufs=4) as qpool,  # qp_dc fp16 1KB
            tc.tile_pool(name="xqp", bufs=1) as xqp,      # xqT fp16 4KB
            tc.tile_pool(name="s16p", bufs=4) as s16p,    # S fp16 1KB
            tc.tile_pool(name="st16p", bufs=4) as st16p,  # S^T fp16 1KB
            tc.tile_pool(name="osb", bufs=4) as osb,
            tc.tile_pool(name="pS", bufs=2, space="PSUM") as pS,
            tc.tile_pool(name="pT", bufs=2, space="PSUM") as pT,
            tc.tile_pool(name="pSt", bufs=2, space="PSUM") as pSt,
        ):
            # ---- persistent buffers ----
            ident = pp.tile([128, 128], F32, tag="ident")
            nc.sync.dma_start(ident[:, :], ident_in.ap())
            sel = pp.tile([128, 2], F32, tag="sel")
            nc.sync.dma_start(sel[:, :], sel_in.ap())
            selb = pp.tile([AB, NLC * 128], F32, tag="selb")
            nc.sync.dma_start(selb[:, :], selb_in.ap())
            bq_sb = pp.tile([128, NCC], F32, tag="bq")
            nc.sync.dma_start(bq_sb[:, :], bq4.ap())
            bk_sb = pp.tile([128, NCC], F32, tag="bk")
            nc.sync.dma_start(bk_sb[:, :], bk4.ap())
            ls_sb = pp.tile([128, 1], F32, tag="ls")
            nc.sync.dma_start(ls_sb[:, :], ls128.ap())
            mask_sb = pp.tile([AB, L], F32, tag="mask")
            nc.sync.dma_start(mask_sb[:, :], mask16.ap())

            gT = pp.tile([128, NCC * LQ], MMDT, tag="gT")
            xkT = pp.tile([128, NCC * MK], MMDT, tag="xkT")
            xkT_v = xkT[:, :].rearrange("p (cc m) -> p cc m", cc=NCC)
            h_col = pp.tile([128, NLC], F32, tag="hcol")
            recip_l = pp.tile([128, NLC], F32, tag="recipl")
            sel_scaled = pp.tile([128, 2], F32, tag="selsc")
            t2v_buf = pp.tile([128, NLC * 128], F32, tag="t2v")
            v2t_buf = pp.tile([128, NLC * NMT * 8], F32, tag="v2t")
            wk16 = pp.tile([128, NCC * D], MMDT, tag="wk16")
            ident16 = pp.tile([128, 128], F16, tag="ident16")
            nc.vector.tensor_copy(ident16[:, :], ident[:, :])
            wqT = pp.tile([128, NCC * D], MMDT, tag="wqT")
            bk16 = pp.tile([128, NCC], MMDT, tag="bk16")

            # ---- small scalar prep ----
            expls = pp.tile([128, 1], F32, tag="expls")
            nc.scalar.activation(expls[:, :], ls_sb[:, :], AF.Exp)
            half_expls = pp.tile([128, 1], F32, tag="hexpls")
            nc.scalar.mul(half_expls[:, :], expls[:, :], 0.5)
            v2t_scale = pp.tile([128, 1], F32, tag="v2tscale")
            nc.scalar.mul(v2t_scale[:, :], expls[:, :], 1.0 / (2.0 * L))
            nc.vector.tensor_scalar_mul(sel_scaled[:, :], sel[:, :],
                                        v2t_scale[:, 0:1])
            msum = pp.tile([AB, 1], F32, tag="msum")
            nc.vector.reduce_sum(msum[:, :], mask_sb[:, :], axis=AX.X)
            mrec = pp.tile([AB, 1], F32, tag="mrec")
            nc.vector.reciprocal(mrec[:, :], msum[:, :])
            ps_r = pT.tile([128, NLC], F32, tag="tk")
            for lc in range(NLC):
                nc.tensor.matmul(ps_r[:, lc:lc + 1],
                                 selb[:, lc * 128:lc * 128 + 128],
                                 mrec[:, 0:1],
                                 start=True, stop=True)
            # recip_l includes the exp(ls)/2 factor
            nc.vector.tensor_scalar_mul(recip_l[:, :], ps_r[:, :],
                                        half_expls[:, 0:1])
            nc.vector.tensor_copy(bk16[:, :], bk_sb[:, :])

            # ---- wk: load fp32, convert to fp16 ----
            for dc in range(NCC):
                st = stg.tile([128, D], F32, tag="stg")
                nc.sync.dma_start(st[:, :], wk.ap()[dc * 128:dc * 128 + 128, :])
                nc.scalar.copy(wk16[:, dc * D:(dc + 1) * D], st[:, :])

            # ---- WqT: wqT[p, cc, d] = Wq[d, cc*128+p] (fp16) ----
            for dc in range(NCC):
                st = stg.tile([128, D], F32, tag="stg")
                nc.sync.dma_start(st[:, :], wq.ap()[dc * 128:dc * 128 + 128, :])
                ps = pT.tile([128, 512], F32, tag="tk")
                for cc in range(NCC):
                    nc.tensor.transpose(ps[:, cc * 128:cc * 128 + 128],
                                        st[:, cc * 128:cc * 128 + 128],
                                        ident[:, :])
                nc.scalar.copy(
                    wqT[:, :].rearrange("p (cc d) -> p cc d", cc=NCC)
                    [:, :, dc * 128:dc * 128 + 128],
                    ps[:, :].rearrange("p (cc d) -> p cc d", cc=NCC))

            # ---- q-side: QpT per dc on the fly; G and h ----
            ps_h = pT.tile([128, NLC], F32, tag="tk")
            for lt in range(NLT):
                xqT = xqp.tile([128, NCC * 512], MMDT, tag="xqT",
                               name=f"xqT_{lt}")
                for j in range(4):
                    rc = lt * 4 + j
                    st = stg.tile([128, D], F32, tag="stg")
                    nc.sync.dma_start(st[:, :],
                                      xq.ap()[rc * 128:rc * 128 + 128, :])
                    ps = pT.tile([128, 512], F32, tag="tk")
                    for cc in range(NCC):
                        nc.tensor.transpose(
                            ps[:, cc * 128:cc * 128 + 128],
                            st[:, cc * 128:cc * 128 + 128], ident[:, :])
                    nc.scalar.copy(
                        xqT[:, :].rearrange("p (cc l) -> p cc l", cc=NCC)
                        [:, :, j * 128:j * 128 + 128],
                        ps[:, :].rearrange("p (cc l) -> p cc l", cc=NCC))

                qp_tiles = []
                for dc in range(NCC):
                    ps_q = pS.tile([128, 512], F32, tag="s0")
                    for cc in range(NCC):
                        nc.tensor.matmul(
                            ps_q[:, :],
                            wqT[:, cc * D + dc * 128:cc * D + dc * 128 + 128],
                            xqT[:, cc * 512:(cc + 1) * 512],
                            start=(cc == 0), stop=(cc == NCC - 1))
                    qp_dc = qpool.tile([128, 512], MMDT, tag="qp",
                                       name=f"qp_{lt}_{dc}")
                    nc.scalar.activation(qp_dc[:, :], ps_q[:, :], AF.Identity,
                                         bias=bq_sb[:, dc:dc + 1])
                    qp_tiles.append(qp_dc)
                # G^T: cc-outer, dc-inner accumulation
                for cc in range(NCC):
                    ps_g = pT.tile([128, 512], F32, tag="tk",
                                   name=f"ps_g_{lt}_{cc}")
                    for dc in range(NCC):
                        nc.tensor.matmul(
                            ps_g[:, :],
                            wk16[:, dc * D + cc * 128:dc * D + cc * 128 + 128],
                            qp_tiles[dc][:, :],
                            start=(dc == 0), stop=(dc == NCC - 1))
                    nc.scalar.copy(
                        gT[:, cc * LQ + lt * 512:cc * LQ + lt * 512 + 512],
                        ps_g[:, :])
                # h for the 4 l-chunks of this lt
                for lj in range(4):
                    lc = lt * 4 + lj
                    for dc in range(NCC):
                        nc.tensor.matmul(
                            ps_h[:, lc:lc + 1],
                            qp_tiles[dc][:, lj * 128:lj * 128 + 128],
                            bk16[:, dc:dc + 1],
                            start=(dc == 0), stop=(dc == NCC - 1))
            nc.vector.tensor_copy(h_col[:, :], ps_h[:, :])

            # ---- XkT build: gpsimd cast-DMA to fp16, fp16 PE transpose ----
            for rc in range(MK // 128):
                st16 = stg.tile([128, D], MMDT, tag="stg16")
                nc.gpsimd.dma_start(st16[:, :],
                                    xk.ap()[rc * 128:rc * 128 + 128, :])
                ps = pSt.tile([128, 512], F16, tag="st", name=f"ps_xk_{rc}")
                for cc in range(NCC):
                    nc.tensor.transpose(ps[:, cc * 128:cc * 128 + 128],
                                        st16[:, cc * 128:cc * 128 + 128],
                                        ident16[:, :])
                nc.scalar.copy(
                    xkT_v[:, :, rc * 128:rc * 128 + 128],
                    ps[:, :].rearrange("p (cc m) -> p cc m", cc=NCC))

            # ---- main loop (paired m-tiles) ----
            for rep in range(repeat_main):
                for lc in range(NLC):
                    for mtp in range(NMT // 2):
                        ps_s = pS.tile([128, 1024], F32, tag="s0")
                        for half in range(2):
                            mt = mtp * 2 + half
                            for cc in range(NCC):
                                nc.tensor.matmul(
                                    ps_s[:, half * 512:half * 512 + 512],
                                    gT[:, cc * LQ + lc * 128:
                                       cc * LQ + lc * 128 + 128],
                                    xkT_v[:, cc, mt * 512:mt * 512 + 512],
                                    start=(cc == 0), stop=(cc == NCC - 1))
                        # t2v: max over m within 64-groups (h added later)
                        if "t2v" not in ablate:
                            nc.vector.reduce_max(
                                t2v_buf[:, lc * 128 + mtp * 16:
                                        lc * 128 + mtp * 16 + 16],
                                ps_s[:, :].rearrange("p (g k) -> p g k", k=L),
                                axis=AX.X)
                        if "evict" in ablate:
                            continue
                        s16 = s16p.tile([128, 1024], MMDT, tag="s16")
                        nc.scalar.activation(s16[:, :], ps_s[:, :],
                                             AF.Identity,
                                             bias=h_col[:, lc:lc + 1])
                        if "v2t" in ablate:
                            continue
                        ps_t = pSt.tile([128, 1024], F16, tag="st")
                        for q in range(8):
                            nc.tensor.transpose(
                                ps_t[:, q * 128:q * 128 + 128],
                                s16[:, q * 128:q * 128 + 128],
                                ident16[:, :])
                        nc.vector.reduce_max(
                            v2t_buf[:, (lc * NMT + mtp * 2) * 8:
                                    (lc * NMT + mtp * 2) * 8 + 16]
                            .rearrange("p (q g) -> p q g", q=8),
                            ps_t[:, :].rearrange("p (q g k) -> p q g k",
                                                 q=8, g=2),
                            axis=AX.X)

            # ---- epilogue: t2v ----
            for lc in range(NLC if ("t2v" not in ablate
                                    and "evict" not in ablate) else 0):
                # t2v_final = (max0 + h) * (exp(ls)/2/cms)
                nc.vector.tensor_scalar(
                    t2v_buf[:, lc * 128:(lc + 1) * 128],
                    t2v_buf[:, lc * 128:(lc + 1) * 128],
                    h_col[:, lc:lc + 1], recip_l[:, lc:lc + 1],
                    op0=ALU.add, op1=ALU.mult)
                ps_o = pT.tile([2, 128], F32, tag="tk")
                nc.tensor.matmul(ps_o[:, :], sel[:, :],
                                 t2v_buf[:, lc * 128:(lc + 1) * 128],
                                 start=True, stop=True)
                o_sb = osb.tile([2, 128], F32, tag="osbt")
                nc.scalar.copy(o_sb[:, :], ps_o[:, :])
                nc.sync.dma_start(out_t2v.ap()[2 * lc:2 * lc + 2, :],
                                  o_sb[:, :])

            # ---- epilogue: v2t ----
            for hv in range(2 if ("v2t" not in ablate
                                  and "evict" not in ablate) else 0):
                ps_o = pT.tile([2, 512], F32, tag="tk")
                nc.tensor.matmul(ps_o[:, :], sel_scaled[:, :],
                                 v2t_buf[:, hv * 512:hv * 512 + 512],
                                 start=True, stop=True)
                o_sb = osb.tile([2, 512], F32, tag="osbv")
                nc.scalar.copy(o_sb[:, :], ps_o[:, :])
                nc.sync.dma_start(out_v2t.ap()[:, hv * 512:hv * 512 + 512],
                                  o_sb[:, :])

    nc.compile()
    return nc


def make_host_inputs(inputs):
    """Split full inputs into 8 per-core in_maps. inputs: dict of np arrays."""
    import numpy as np

    Xq = np.ascontiguousarray(inputs["query_states"], dtype=np.float32)
    Xk = np.ascontiguousarray(inputs["key_states"], dtype=np.float32)
    mask = np.ascontiguousarray(inputs["attention_mask"], dtype=np.float32)
    Wq = np.ascontiguousarray(inputs["Wq"], dtype=np.float32)
    Wk = np.ascontiguousarray(inputs["Wk"], dtype=np.float32)
    bq = np.asarray(inputs["bq"], dtype=np.float32)
    bk = np.asarray(inputs["bk"], dtype=np.float32)
    ls = np.float32(np.asarray(inputs["logit_scale"]))

    bq4 = np.ascontiguousarray(bq.reshape(NCC, 128).T)
    bk4 = np.ascontiguousarray(bk.reshape(NCC, 128).T)
    ls128 = np.full((128, 1), ls, np.float32)
    ident = np.eye(128, dtype=np.float32)
    sel = np.zeros((128, 2), np.float32)
    sel[:64, 0] = 1.0
    sel[64:, 1] = 1.0
    # selb[a, lc*128+p] = 1 iff a == 2*lc + p//64  (recip_l broadcast matmul)
    selb = np.zeros((AB, NLC * 128), np.float32)
    for lc in range(NLC):
        for p in range(128):
            selb[2 * lc + p // 64, lc * 128 + p] = 1.0
    xk2 = np.ascontiguousarray(Xk.reshape(MK, D))

    in_maps = []
    for i in range(N_CORES):
        in_maps.append({
            "xq": np.ascontiguousarray(
                Xq[i * AB:(i + 1) * AB].reshape(LQ, D)),
            "xk": xk2,
            "wq": Wq, "wk": Wk,
            "bq4": bq4, "bk4": bk4,
            "mask16": np.ascontiguousarray(mask[i * AB:(i + 1) * AB]),
            "ls128": ls128, "ident": ident, "sel": sel, "selb": selb,
        })
    return in_maps


def assemble_output(results):
    """results: list of 8 dicts with out_t2v [16,128], out_v2t [2, 1024]."""
    import numpy as np

    r = np.empty((NB, NB), np.float32)
    for i, res in enumerate(results):
        t2v = res["out_t2v"]  # [16, 128] : a_local, b
        v2t = res["out_v2t"].reshape(2, NLC, NMT, 4, 2)  # [half,lc,mt,q,g]
        # a_local = 2*lc+g ; b = mt*8 + q*2 + half
        v2t_ab = v2t.transpose(1, 4, 2, 3, 0).reshape(AB, NB)
        r[i * AB:(i + 1) * AB] = t2v + v2t_ab
    return r, np.ascontiguousarray(r.T)


# ======================= harness entry point =======================

_NC_CACHE = {}


def _get_nc():
    if "nc" not in _NC_CACHE:
        _NC_CACHE["nc"] = build_kernel()
    return _NC_CACHE["nc"]


def kernel(**inputs):
    """Full-input entry point: shards across 8 NeuronCores, runs the Bass
    kernel via PJRT SPMD, gathers per-core partial outputs, and assembles
    the full (r, r.T) result matching the reference."""
    from concourse.bass_utils import run_bass_kernel_spmd

    nc = _get_nc()
    in_maps = make_host_inputs(inputs)
    res = run_bass_kernel_spmd(nc, in_maps, core_ids=list(range(N_CORES)))
    return assemble_output(res.results)



# revision 2
# speedup vs baseline: 38260.9519x; 38260.9519x over previous
"""ContraAttention TRN2 kernel (v3: fp8e4 DoubleRow matmuls, host-prepped
transposed/cast inputs, fp16 packed reduces).

Per-core program (core i owns query batches [16i, 16i+16)):
  QpT = Wq Xq^T + bq ; GT = Wk^T QpT (so G = Qp Wk) ; h = Qp @ bk
  S = G @ Xk^T + h*1^T   (exact: == Qp @ (Xk Wk^T + bk)^T)
  per (a,b) 64x64 block: t2v_sum = sum_l max_m (S), v2t_sum = sum_m max_l S
  r[a,b] = exp(ls) * (t2v_sum/cms[a] + v2t_sum/64) / 2

Host passes (per core): XqT fp16 [512,1024]; shared: WqT fp16, Wk fp16,
XkT fp8e4 [512,8192], biases, ident16, selectors.

Main loop (lc x mtp = 8x8 iterations over the (1024 l, 8192 m) S matrix):
  ps_s[128,1024] = 2 halves x 2 DoubleRow fp8 matmuls (256-deep each)
  s16 = fp16(ps_s + h[l])          (ACT, PSUM->SBUF)
  t2v16 += reduce_max over m-64-groups of s16      (DVE, packed)
  ps_t = PE-transpose(s16) fp16 -> v2t16 reduce over l-64-groups (DVE)

Outputs per core:
  out_t2v [16, 128]  : exp(ls)/2/cms[a] * t2v_sum  at [a_local, b]
  out_v2t [2, 1024]  : exp(ls)/128 * v2t_sum at [half, ((lc*16+mt)*4+q)*2+g]
                       contributing to a_local=2*lc+g, b=mt*8+q*2+half
"""

import sys

sys.path.insert(0, "/opt/trn_rl_repo")

import concourse.bass as bass  # noqa: F401
import concourse.mybir as mybir
import concourse.tile as tile
from concourse import bacc

F32 = mybir.dt.float32
F16 = mybir.dt.float16
F8 = mybir.dt.float8e4
AF = mybir.ActivationFunctionType
AX = mybir.AxisListType
ALU = mybir.AluOpType
DR = mybir.MatmulPerfMode.DoubleRow

N_CORES = 8
NB = 128            # global batches
AB = NB // N_CORES  # 16 batches per core
L = 64              # Lq = Lk
D = 512
LQ = AB * L         # 1024 q rows per core
MK = NB * L         # 8192 k rows
NLC = LQ // 128     # 8 l-chunks
NMT = MK // 512     # 16 m-tiles
NCC = D // 128      # 4 contraction chunks
NLT = LQ // 512     # 2 l-tiles


def build_kernel(repeat_main=1, ablate=()):
    nc = bacc.Bacc("TRN2", target_bir_lowering=False, debug=False,
                   num_devices=N_CORES)

    xqT = nc.dram_tensor("xqT", [D, LQ], F16, kind="ExternalInput")
    wqT = nc.dram_tensor("wqT", [D, D], F16, kind="ExternalInput")
    wk = nc.dram_tensor("wk", [D, D], F16, kind="ExternalInput")
    xk8 = nc.dram_tensor("xk8", [D, MK], F8, kind="ExternalInput")
    bq4 = nc.dram_tensor("bq4", [128, NCC], F32, kind="ExternalInput")
    bk4 = nc.dram_tensor("bk4", [128, NCC], F16, kind="ExternalInput")
    mask16 = nc.dram_tensor("mask16", [AB, L], F32, kind="ExternalInput")
    ls128 = nc.dram_tensor("ls128", [128, 1], F32, kind="ExternalInput")
    ident_in = nc.dram_tensor("ident", [128, 128], F16, kind="ExternalInput")
    sel_in = nc.dram_tensor("sel", [128, 2], F16, kind="ExternalInput")
    selb_in = nc.dram_tensor("selb", [AB, NLC * 128], F32,
                             kind="ExternalInput")

    out_t2v = nc.dram_tensor("out_t2v", [AB, 128], F32, kind="ExternalOutput")
    out_v2t = nc.dram_tensor("out_v2t", [2, NLC * NMT * 8], F32,
                             kind="ExternalOutput")

    with tile.TileContext(nc) as tc:
        with (
            tc.tile_pool(name="persist", bufs=1) as pp,
            tc.tile_pool(name="qpool", bufs=4) as qpool,  # qp fp16 1KB
            tc.tile_pool(name="s16p", bufs=4) as s16p,    # S fp16 2KB
            tc.tile_pool(name="osb", bufs=4) as osb,
            tc.tile_pool(name="pS", bufs=2, space="PSUM") as pS,
            tc.tile_pool(name="pT", bufs=2, space="PSUM") as pT,
            tc.tile_pool(name="pSt", bufs=2, space="PSUM") as pSt,
        ):
            # ---- persistent buffers ----
            ident16 = pp.tile([128, 128], F16, tag="ident16")
            nc.sync.dma_start(ident16[:, :], ident_in.ap())
            sel16 = pp.tile([128, 2], F16, tag="sel16")
            nc.sync.dma_start(sel16[:, :], sel_in.ap())
            selb = pp.tile([AB, NLC * 128], F32, tag="selb")
            nc.sync.dma_start(selb[:, :], selb_in.ap())
            bq_sb = pp.tile([128, NCC], F32, tag="bq")
            nc.sync.dma_start(bq_sb[:, :], bq4.ap())
            bk_sb = pp.tile([128, NCC], F16, tag="bk")
            nc.sync.dma_start(bk_sb[:, :], bk4.ap())
            ls_sb = pp.tile([128, 1], F32, tag="ls")
            nc.sync.dma_start(ls_sb[:, :], ls128.ap())
            mask_sb = pp.tile([AB, L], F32, tag="mask")
            nc.sync.dma_start(mask_sb[:, :], mask16.ap())

            wqT_sb = pp.tile([128, NCC * D], F16, tag="wqT")
            wk_sb = pp.tile([128, NCC * D], F16, tag="wk")
            xqT_sb = pp.tile([128, NCC * LQ], F16, tag="xqT")
            xk_sb = pp.tile([128, NCC * MK], F8, tag="xk8")
            for cc in range(NCC):
                nc.sync.dma_start(wqT_sb[:, cc * D:(cc + 1) * D],
                                  wqT.ap()[cc * 128:cc * 128 + 128, :])
                nc.sync.dma_start(wk_sb[:, cc * D:(cc + 1) * D],
                                  wk.ap()[cc * 128:cc * 128 + 128, :])
                nc.sync.dma_start(xqT_sb[:, cc * LQ:(cc + 1) * LQ],
                                  xqT.ap()[cc * 128:cc * 128 + 128, :])
                nc.sync.dma_start(xk_sb[:, cc * MK:(cc + 1) * MK],
                                  xk8.ap()[cc * 128:cc * 128 + 128, :])

            gT8 = pp.tile([128, NCC * LQ], F8, tag="gT8")
            h_col = pp.tile([128, NLC], F32, tag="hcol")
            recip_l = pp.tile([128, NLC], F32, tag="recipl")
            sel_scaled = pp.tile([128, 2], F16, tag="selsc")
            t2v16 = pp.tile([128, NLC * 128], F16, tag="t2v")
            v2t16 = pp.tile([128, NLC * NMT * 8], F16, tag="v2t")

            # ---- small scalar prep ----
            expls = pp.tile([128, 1], F32, tag="expls")
            nc.scalar.activation(expls[:, :], ls_sb[:, :], AF.Exp)
            half_expls = pp.tile([128, 1], F32, tag="hexpls")
            nc.scalar.mul(half_expls[:, :], expls[:, :], 0.5)
            v2t_scale = pp.tile([128, 1], F32, tag="v2tscale")
            nc.scalar.mul(v2t_scale[:, :], expls[:, :], 1.0 / (2.0 * L))
            nc.vector.tensor_scalar_mul(sel_scaled[:, :], sel16[:, :],
                                        v2t_scale[:, 0:1])
            msum = pp.tile([AB, 1], F32, tag="msum")
            nc.vector.reduce_sum(msum[:, :], mask_sb[:, :], axis=AX.X)
            mrec = pp.tile([AB, 1], F32, tag="mrec")
            nc.vector.reciprocal(mrec[:, :], msum[:, :])
            ps_r = pT.tile([128, NLC], F32, tag="tk")
            for lc in range(NLC):
                nc.tensor.matmul(ps_r[:, lc:lc + 1],
                                 selb[:, lc * 128:lc * 128 + 128],
                                 mrec[:, 0:1],
                                 start=True, stop=True)
            # recip_l includes the exp(ls)/2 factor
            nc.vector.tensor_scalar_mul(recip_l[:, :], ps_r[:, :],
                                        half_expls[:, 0:1])

            # ---- q-side: QpT per dc; G^T (fp8) and h ----
            ps_h = pT.tile([128, NLC], F32, tag="tk")
            for lt in range(NLT):
                qp_tiles = []
                for dc in range(NCC):
                    ps_q = pS.tile([128, 512], F32, tag="s0")
                    for cc in range(NCC):
                        nc.tensor.matmul(
                            ps_q[:, :],
                            wqT_sb[:, cc * D + dc * 128:cc * D + dc * 128 + 128],
                            xqT_sb[:, cc * LQ + lt * 512:cc * LQ + lt * 512 + 512],
                            start=(cc == 0), stop=(cc == NCC - 1))
                    qp_dc = qpool.tile([128, 512], F16, tag="qp",
                                       name=f"qp_{lt}_{dc}")
                    nc.scalar.activation(qp_dc[:, :], ps_q[:, :], AF.Identity,
                                         bias=bq_sb[:, dc:dc + 1])
                    qp_tiles.append(qp_dc)
                # G^T: cc-outer, dc-inner accumulation; store fp8
                for cc in range(NCC):
                    ps_g = pT.tile([128, 512], F32, tag="tk",
                                   name=f"ps_g_{lt}_{cc}")
                    for dc in range(NCC):
                        nc.tensor.matmul(
                            ps_g[:, :],
                            wk_sb[:, dc * D + cc * 128:dc * D + cc * 128 + 128],
                            qp_tiles[dc][:, :],
                            start=(dc == 0), stop=(dc == NCC - 1))
                    nc.scalar.copy(
                        gT8[:, cc * LQ + lt * 512:cc * LQ + lt * 512 + 512],
                        ps_g[:, :])
                # h for the 4 l-chunks of this lt
                for lj in range(4):
                    lc = lt * 4 + lj
                    for dc in range(NCC):
                        nc.tensor.matmul(
                            ps_h[:, lc:lc + 1],
                            qp_tiles[dc][:, lj * 128:lj * 128 + 128],
                            bk_sb[:, dc:dc + 1],
                            start=(dc == 0), stop=(dc == NCC - 1))
            nc.vector.tensor_copy(h_col[:, :], ps_h[:, :])

            # DoubleRow views: planes (cc2, i) with i-stride = one cc chunk
            gT8_v = gT8[:, :].rearrange("p (cc2 i l) -> p cc2 i l",
                                        cc2=2, i=2)
            xk8_v = xk_sb[:, :].rearrange("p (cc2 i m) -> p cc2 i m",
                                          cc2=2, i=2)

            # ---- main loop (paired m-tiles) ----
            for rep in range(repeat_main):
                for lc in range(NLC):
                    for mtp in range(NMT // 2):
                        ps_s = pS.tile([128, 1024], F32, tag="s0")
                        for half in range(2):
                            mt = mtp * 2 + half
                            for cc2 in range(2):
                                nc.tensor.matmul(
                                    ps_s[:, half * 512:half * 512 + 512],
                                    gT8_v[:, cc2, :,
                                          lc * 128:lc * 128 + 128],
                                    xk8_v[:, cc2, :,
                                          mt * 512:mt * 512 + 512],
                                    start=(cc2 == 0), stop=(cc2 == 1),
                                    perf_mode=DR)
                        if "evict" in ablate:
                            continue
                        s16 = s16p.tile([128, 1024], F16, tag="s16")
                        nc.scalar.activation(s16[:, :], ps_s[:, :],
                                             AF.Identity,
                                             bias=h_col[:, lc:lc + 1])
                        # t2v: max over m within 64-groups (h included)
                        if "t2v" not in ablate:
                            nc.vector.reduce_max(
                                t2v16[:, lc * 128 + mtp * 16:
                                      lc * 128 + mtp * 16 + 16],
                                s16[:, :].rearrange("p (g k) -> p g k", k=L),
                                axis=AX.X)
                        if "v2t" in ablate:
                            continue
                        ps_t = pSt.tile([128, 1024], F16, tag="st")
                        for q in range(8):
                            nc.tensor.transpose(
                                ps_t[:, q * 128:q * 128 + 128],
                                s16[:, q * 128:q * 128 + 128],
                                ident16[:, :])
                        nc.vector.reduce_max(
                            v2t16[:, (lc * NMT + mtp * 2) * 8:
                                  (lc * NMT + mtp * 2) * 8 + 16]
                            .rearrange("p (q g) -> p q g", q=8),
                            ps_t[:, :].rearrange("p (q g k) -> p q g k",
                                                 q=8, g=2),
                            axis=AX.X)

            # ---- epilogue: t2v ----
            for lc in range(NLC if ("t2v" not in ablate
                                    and "evict" not in ablate) else 0):
                # t2v_final = max16 * (exp(ls)/2/cms)   (h already in s16)
                nc.vector.tensor_scalar_mul(
                    t2v16[:, lc * 128:(lc + 1) * 128],
                    t2v16[:, lc * 128:(lc + 1) * 128],
                    recip_l[:, lc:lc + 1])
                ps_o = pT.tile([2, 128], F32, tag="tk")
                nc.tensor.matmul(ps_o[:, :], sel16[:, :],
                                 t2v16[:, lc * 128:(lc + 1) * 128],
                                 start=True, stop=True)
                o_sb = osb.tile([2, 128], F32, tag="osbt")
                nc.scalar.copy(o_sb[:, :], ps_o[:, :])
                nc.sync.dma_start(out_t2v.ap()[2 * lc:2 * lc + 2, :],
                                  o_sb[:, :])

            # ---- epilogue: v2t ----
            for hv in range(2 if ("v2t" not in ablate
                                  and "evict" not in ablate) else 0):
                ps_o = pT.tile([2, 512], F32, tag="tk")
                nc.tensor.matmul(ps_o[:, :], sel_scaled[:, :],
                                 v2t16[:, hv * 512:hv * 512 + 512],
                                 start=True, stop=True)
                o_sb = osb.tile([2, 512], F32, tag="osbv")
                nc.scalar.copy(o_sb[:, :], ps_o[:, :])
                nc.sync.dma_start(out_v2t.ap()[:, hv * 512:hv * 512 + 512],
                                  o_sb[:, :])

    nc.compile()
    return nc


def make_host_inputs(inputs):
    """Split full inputs into 8 per-core in_maps. inputs: dict of np arrays."""
    import numpy as np

    f8np = mybir.dt.np(F8)

    Xq = np.ascontiguousarray(inputs["query_states"], dtype=np.float32)
    Xk = np.ascontiguousarray(inputs["key_states"], dtype=np.float32)
    mask = np.ascontiguousarray(inputs["attention_mask"], dtype=np.float32)
    Wq = np.ascontiguousarray(inputs["Wq"], dtype=np.float32)
    Wk = np.ascontiguousarray(inputs["Wk"], dtype=np.float32)
    bq = np.asarray(inputs["bq"], dtype=np.float32)
    bk = np.asarray(inputs["bk"], dtype=np.float32)
    ls = np.float32(np.asarray(inputs["logit_scale"]))

    bq4 = np.ascontiguousarray(bq.reshape(NCC, 128).T)
    bk4 = np.ascontiguousarray(bk.reshape(NCC, 128).T).astype(np.float16)
    ls128 = np.full((128, 1), ls, np.float32)
    ident = np.eye(128, dtype=np.float16)
    sel = np.zeros((128, 2), np.float16)
    sel[:64, 0] = 1.0
    sel[64:, 1] = 1.0
    # selb[a, lc*128+p] = 1 iff a == 2*lc + p//64  (recip_l broadcast matmul)
    selb = np.zeros((AB, NLC * 128), np.float32)
    for lc in range(NLC):
        for p in range(128):
            selb[2 * lc + p // 64, lc * 128 + p] = 1.0

    wqT16 = np.ascontiguousarray(Wq.T, dtype=np.float16)
    wk16 = Wk.astype(np.float16)
    xk8 = np.ascontiguousarray(
        Xk.reshape(MK, D).T).astype(np.float16).astype(f8np)

    in_maps = []
    for i in range(N_CORES):
        xqT16 = np.ascontiguousarray(
            Xq[i * AB:(i + 1) * AB].reshape(LQ, D).T, dtype=np.float16)
        in_maps.append({
            "xqT": xqT16,
            "xk8": xk8,
            "wqT": wqT16, "wk": wk16,
            "bq4": bq4, "bk4": bk4,
            "mask16": np.ascontiguousarray(mask[i * AB:(i + 1) * AB]),
            "ls128": ls128, "ident": ident, "sel": sel, "selb": selb,
        })
    return in_maps


def assemble_output(results):
    """results: list of 8 dicts with out_t2v [16,128], out_v2t [2, 1024]."""
    import numpy as np

    r = np.empty((NB, NB), np.float32)
    for i, res in enumerate(results):
        t2v = res["out_t2v"]  # [16, 128] : a_local, b
        v2t = res["out_v2t"].reshape(2, NLC, NMT, 4, 2)  # [half,lc,mt,q,g]
        # a_local = 2*lc+g ; b = mt*8 + q*2 + half
        v2t_ab = v2t.transpose(1, 4, 2, 3, 0).reshape(AB, NB)
        r[i * AB:(i + 1) * AB] = t2v + v2t_ab
    return r, np.ascontiguousarray(r.T)


# ======================= harness entry point =======================

_NC_CACHE = {}


def _get_nc():
    if "nc" not in _NC_CACHE:
        _NC_CACHE["nc"] = build_kernel()
    return _NC_CACHE["nc"]


def kernel(**inputs):
    """Full-input entry point: shards across 8 NeuronCores, runs the Bass
    kernel via PJRT SPMD, gathers per-core partial outputs, and assembles
    the full (r, r.T) result matching the reference."""
    from concourse.bass_utils import run_bass_kernel_spmd

    nc = _get_nc()
    in_maps = make_host_inputs(inputs)
    res = run_bass_kernel_spmd(nc, in_maps, core_ids=list(range(N_CORES)))
    return assemble_output(res.results)
